# revision 28
# baseline (speedup 1.0000x reference)
"""VQ codebook quantizer for Trainium2, 8-core data-parallel.

x: (8, 2048, 512) f32, codebook: (8192, 512) f32.
Per core: 2048 tokens. scores[t,k] = 2*x@e.T - ||e||^2 (argmax == argmin dist;
||x||^2 dropped as argmin-invariant).

Wall-clock layout (the axon tunnel runs at ~75 MB/s with ~70 ms RTT, so bytes
moved per call dominate): the jitted executable and all codebook-derived
tensors (et, ne2, sel, ident; ~128 MB replicated over 8 cores) are
built/uploaded ONCE and kept device-resident; per call only x goes up -- as
bf16 (16 MB instead of 32) -- and per-token results come back (256 KB):
top-1/top-2 argmin candidates plus the top1-top2 and top1-top3 score gaps.
bf16 rounding perturbs each score by sigma ~ 0.058, so tokens with
gap12 < ~8.6 sigma (~3%) are re-decided on host by exact f32 scores of just
the two candidates (~1 ms); the handful with gap13 also tiny get a full
exact rescan. This restores the exact f32 argmin. The donated
output buffer is created on-device by a tiny jitted zeros fn (a host-side
64 KB device_put costs a full 74 ms RTT). Host does the final codebook[codes]
row lookup.

On-device: x tile loads in natural [token, d] bf16 layout, DVE-converts to
f32, PE-transposes (is_transpose matmul) into x^T chunks. Per (t_tile,
k_chunk): 4 accumulating fp32 matmuls with lhsT = x^T tile, rhs = (2e)^T
chunk, plus a 5th rank-16 matmul broadcasting -||e||^2 via a one-hot weight.
ACT evacuates PSUM->SBUF; DVE max8/max_index per 512-chunk; DVE merge
(max8 over chunk maxima + is_ge + select + reduce_min for first-occurrence
ties) yields the argmin code and the true global top1-top2 gap per token
(exact-tie across chunks => gap 0 => flagged, so ties are safe).
"""

import numpy as np

N_CORES = 8
B, S, D = 8, 2048, 512
K = 8192
N_PER_CORE = (B * S) // N_CORES  # 2048
T_TILES = N_PER_CORE // 128  # 16
KC = K // 512  # 16 chunks of 512 codes
DC = D // 128  # 4 contraction chunks

# bf16(x) perturbs each score by sigma ~ 0.058 (pairwise); tokens whose
# top1-top2 gap is under GAP_THETA (~8.6 sigma) get an exact host re-check
# between the device's top-2 candidates; tokens whose top1-top3 gap is also
# under it (a handful) get a full exact rescan.
GAP_THETA = 0.5

_CACHED = {}


def build_nc():
    import concourse.bacc as bacc
    import concourse.mybir as mybir
    from concourse.tile import TileContext

    f32 = mybir.dt.float32
    bf16 = mybir.dt.bfloat16
    u16 = mybir.dt.uint16

    nc = bacc.Bacc("TRN2", target_bir_lowering=False, debug=False,
                   num_devices=N_CORES)
    # declaration order == in_names order of the jitted runner
    xn = nc.dram_tensor("xn", [N_PER_CORE, D], bf16, kind="ExternalInput")
    et = nc.dram_tensor("et", [D, K], f32, kind="ExternalInput")  # (2*cb).T
    ne2 = nc.dram_tensor("ne2", [16, 512], f32, kind="ExternalInput")
    seld = nc.dram_tensor("sel", [16, KC * 128], f32, kind="ExternalInput")
    identd = nc.dram_tensor("ident", [128, 128], f32, kind="ExternalInput")
    # global code index of each (chunk, top-3 slot) candidate, host const
    offs3d = nc.dram_tensor("offs3", [128, KC * 3], f32, kind="ExternalInput")
    # T_TILES-col blocks: [idx1, idx2, gap12, gap13]
    out = nc.dram_tensor("out", [128, 4 * T_TILES], f32, kind="ExternalOutput")

    with TileContext(nc) as tc:
        with (
            tc.tile_pool(name="const", bufs=1) as cpool,
            tc.tile_pool(name="xin", bufs=3) as xip,
            tc.tile_pool(name="xcv", bufs=3) as xcp,
            tc.tile_pool(name="xtp", bufs=3) as xtp,
            tc.tile_pool(name="psum", bufs=6, space="PSUM") as pp,
            tc.tile_pool(name="ptr", bufs=2, space="PSUM") as pt,
            tc.tile_pool(name="stage", bufs=6) as sp,
            tc.tile_pool(name="merge", bufs=2) as mp,
            tc.tile_pool(name="fin", bufs=2) as fp_,
        ):
            # --- constants / static loads ---
            ld = nc.sync.dma_start
            et_sb = cpool.tile([128, DC, K], f32)  # 128KB/partition
            ld(et_sb[:], et.rearrange("(dc p) k -> p dc k", p=128))
            ne2_sb = cpool.tile([16, 512], f32)
            ld(ne2_sb[:], ne2[:, :])
            # one-hot row weights: sel[c, kc*128+m] = 1.0 iff c == kc (host const)
            sel = cpool.tile([16, KC * 128], f32)
            ld(sel[:], seld[:, :])
            ident = cpool.tile([128, 128], f32)
            ld(ident[:], identd[:, :])
            offs3 = cpool.tile([128, KC, 3], f32)
            ld(offs3[:], offs3d.rearrange("p (kc s) -> p kc s", s=3))
            # chunk offsets 0,512,...,7680 replicated on every partition
            offs = cpool.tile([128, KC], f32)
            offs_i = cpool.tile([128, KC], mybir.dt.int32)
            nc.gpsimd.iota(offs_i[:], pattern=[[512, KC]], base=0,
                           channel_multiplier=0)
            nc.vector.tensor_copy(offs[:], offs_i[:])
            big = cpool.tile([128, KC], f32)
            nc.vector.memset(big[:], 1e9)
            big3 = cpool.tile([128, KC, 3], f32)
            nc.vector.memset(big3[:], 1e9)
            idx1_all = cpool.tile([128, T_TILES], f32)
            idx2_all = cpool.tile([128, T_TILES], f32)
            g12_all = cpool.tile([128, T_TILES], f32)
            g13_all = cpool.tile([128, T_TILES], f32)

            for t in range(T_TILES):
                # natural-layout bf16 token tile -> f32 -> PE transpose
                xin = xip.tile([128, 512], bf16, tag="xin")
                ld(xin[:], xn[t * 128:(t + 1) * 128, :])
                xcv = xcp.tile([128, 512], f32, tag="xcv")
                nc.vector.tensor_copy(xcv[:], xin[:])
                ps_t = pt.tile([128, 512], f32, tag="pst")
                for dc in range(DC):
                    nc.tensor.transpose(ps_t[:, dc * 128:(dc + 1) * 128],
                                        xcv[:, dc * 128:(dc + 1) * 128],
                                        ident[:])
                xt_sb = xtp.tile([128, 512], f32, tag="xt")
                nc.scalar.copy(xt_sb[:], ps_t[:])

                vals8 = mp.tile([128, KC, 8], f32, tag="v8")
                idx8 = mp.tile([128, KC, 8], u16, tag="i8")
                for kc in range(KC):
                    ps = pp.tile([128, 512], f32, tag="ps")
                    for dc in range(DC):
                        nc.tensor.matmul(
                            ps[:],
                            lhsT=xt_sb[:, dc * 128:(dc + 1) * 128],
                            rhs=et_sb[:, dc, kc * 512:(kc + 1) * 512],
                            start=(dc == 0),
                            stop=False,
                        )
                    nc.tensor.matmul(
                        ps[:],
                        lhsT=sel[:, kc * 128:(kc + 1) * 128],
                        rhs=ne2_sb[:],
                        start=False,
                        stop=True,
                    )
                    st = sp.tile([128, 512], f32, tag="st")
                    nc.scalar.copy(st[:], ps[:])
                    nc.vector.max(out=vals8[:, kc, :], in_=st[:])
                    nc.vector.max_index(out=idx8[:, kc, :],
                                        in_max=vals8[:, kc, :], in_values=st[:])
                # merge: the global top-3 values are contained in the
                # per-chunk top-3s, so max8 over vals8[:, :, 0:3] yields the
                # exact global m1 >= m2 >= m3.
                cand_v = vals8[:, :, 0]   # [128, KC] strided
                v3 = vals8[:, :, 0:3]     # [128, KC, 3]
                c8g = fp_.tile([128, 8], f32, tag="c8")
                nc.vector.max(out=c8g[:], in_=v3)
                m1 = c8g[:, 0:1]
                m2 = c8g[:, 1:2]
                m3 = c8g[:, 2:3]
                nc.vector.tensor_sub(g12_all[:, t:t + 1], m1, m2)
                nc.vector.tensor_sub(g13_all[:, t:t + 1], m1, m3)
                # idx1: first-occurrence (lowest) global index achieving m1
                eq = fp_.tile([128, KC], mybir.dt.uint8, tag="eq")
                nc.vector.tensor_scalar(eq[:], cand_v, m1, None,
                                        op0=mybir.AluOpType.is_ge)
                lidx = fp_.tile([128, KC], f32, tag="li")
                nc.vector.tensor_copy(lidx[:], idx8[:, :, 0])  # u16 -> f32
                nc.vector.tensor_add(lidx[:], lidx[:], offs[:])
                selv = fp_.tile([128, KC], f32, tag="sv")
                nc.vector.select(selv[:], eq[:], lidx[:], big[:])
                nc.vector.tensor_reduce(idx1_all[:, t:t + 1], selv[:],
                                        axis=mybir.AxisListType.X,
                                        op=mybir.AluOpType.min)
                # idx2: lowest global index among (chunk, slot<3) candidates
                # whose value == m2 (exact-duplicate cases land in the full
                # host rescan via g13 == g12)
                i3 = fp_.tile([128, KC, 3], f32, tag="i3")
                nc.vector.tensor_copy(i3[:], idx8[:, :, 0:3])  # u16 -> f32
                nc.vector.tensor_add(i3[:], i3[:], offs3[:])
                eq2 = fp_.tile([128, KC, 3], mybir.dt.uint8, tag="eq2")
                nc.vector.tensor_scalar(eq2[:], v3, m2, None,
                                        op0=mybir.AluOpType.is_equal)
                sel2 = fp_.tile([128, KC, 3], f32, tag="s2")
                nc.vector.select(sel2[:], eq2[:], i3[:], big3[:])
                nc.vector.tensor_reduce(idx2_all[:, t:t + 1], sel2[:],
                                        axis=mybir.AxisListType.XY,
                                        op=mybir.AluOpType.min)

            # ship codes + gaps to DRAM; host does lookup + near-tie verify
            nc.sync.dma_start(out[:, 0:T_TILES], idx1_all[:])
            nc.sync.dma_start(out[:, T_TILES:2 * T_TILES], idx2_all[:])
            nc.sync.dma_start(out[:, 2 * T_TILES:3 * T_TILES], g12_all[:])
            nc.sync.dma_start(out[:, 3 * T_TILES:4 * T_TILES], g13_all[:])

    nc.compile()
    return nc


def _get_runner():
    if "runner" in _CACHED:
        return _CACHED["runner"]

    import jax
    import jax.numpy as jnp
    import concourse.mybir as mybir
    from concourse.bass2jax import (
        _bass_exec_p,
        partition_id_tensor,
        install_neuronx_cc_hook,
        shard_map,
        Mesh,
        PartitionSpec,
    )
    from jax.sharding import NamedSharding

    install_neuronx_cc_hook()
    nc = build_nc()

    partition_name = (nc.partition_id_tensor.name
                      if nc.partition_id_tensor is not None else None)
    in_names, out_names, out_avals = [], [], []
    for alloc in nc.m.functions[0].allocations:
        if not isinstance(alloc, mybir.MemoryLocationSet):
            continue
        name = alloc.memorylocations[0].name
        if alloc.kind == "ExternalInput":
            if name != partition_name:
                in_names.append(name)
        elif alloc.kind == "ExternalOutput":
            shape = tuple(alloc.tensor_shape)
            dtype = mybir.dt.np(alloc.dtype)
            out_names.append(name)
            out_avals.append(jax.core.ShapedArray(shape, dtype))
    n_params = len(in_names)
    n_outs = len(out_avals)
    all_in_names = list(in_names) + list(out_names)
    if partition_name is not None:
        all_in_names.append(partition_name)
    donate = tuple(range(n_params, n_params + n_outs))

    def _body(*args):
        operands = list(args)
        if partition_name is not None:
            operands.append(partition_id_tensor())
        outs = _bass_exec_p.bind(
            *operands,
            out_avals=tuple(out_avals),
            in_names=tuple(all_in_names),
            out_names=tuple(out_names),
            lowering_input_output_aliases=(),
            sim_require_finite=True,
            sim_require_nnan=True,
            nc=nc,
        )
        return tuple(outs)

    devices = jax.devices()[:N_CORES]
    mesh = Mesh(np.asarray(devices), ("core",))
    in_specs = (PartitionSpec("core"),) * (n_params + n_outs)
    out_specs = (PartitionSpec("core"),) * n_outs
    jitted = jax.jit(
        shard_map(_body, mesh=mesh, in_specs=in_specs, out_specs=out_specs,
                  check_rep=False),
        donate_argnums=donate,
        keep_unused=True,
    )
    sharding = NamedSharding(mesh, PartitionSpec("core"))
    # donated output buffer, created on-device (no host->device RTT)
    zeros_fn = jax.jit(
        lambda: jnp.zeros((N_CORES * 128, 4 * T_TILES), jnp.float32),
        out_shardings=sharding,
    )
    from concurrent.futures import ThreadPoolExecutor

    runner = {
        "jitted": jitted,
        "in_names": in_names,
        "zeros_fn": zeros_fn,
        "sharding": sharding,
        "devices": list(devices),
        "pool": ThreadPoolExecutor(N_CORES),
    }
    _CACHED["runner"] = runner
    return runner


def _get_cb_arrays(codebook, runner):
    import jax

    st = _CACHED.get("cb")
    if st is not None and np.array_equal(codebook, st["cb_copy"]):
        return st
    sharding = runner["sharding"]

    def rep(a):
        # replicate across the 8 cores as one global [8*dim0, ...] array
        g = np.ascontiguousarray(
            np.broadcast_to(a, (N_CORES,) + a.shape)
        ).reshape(N_CORES * a.shape[0], *a.shape[1:])
        return jax.device_put(g, sharding)

    cbT2 = np.ascontiguousarray((2.0 * codebook).T)        # [512, 8192]
    e2 = np.sum(codebook * codebook, axis=1, dtype=np.float32)
    selm = np.zeros((16, KC * 128), dtype=np.float32)
    for c in range(KC):
        selm[c, c * 128:(c + 1) * 128] = 1.0
    ident = np.eye(128, dtype=np.float32)
    offs3 = np.broadcast_to(
        np.repeat(np.arange(KC, dtype=np.float32) * 512.0, 3), (128, KC * 3))
    st = {
        "cb_copy": codebook.copy(),
        "et": rep(cbT2),
        "ne2": rep((-e2).reshape(16, 512)),
        "sel": rep(selm),
        "ident": rep(ident),
        "offs3": rep(np.ascontiguousarray(offs3)),
        # host-side exact-verify operands
        "cbT2": cbT2,      # (2*cb).T, f32 contiguous
        "e2": e2,
    }
    for nm in ("et", "ne2", "sel", "ident", "offs3"):
        st[nm].block_until_ready()
    _CACHED["cb"] = st
    return st


def kernel(x: np.ndarray, codebook: np.ndarray) -> np.ndarray:
    import jax
    import ml_dtypes
    import queue as _queue

    runner = _get_runner()
    x = np.asarray(x, dtype=np.float32)
    x_flat = x.reshape(B * S, D)

    # convert + upload per device in parallel threads (the tunnel sustains
    # higher aggregate bandwidth with concurrent per-device transfers).
    # Each thread hands its array handle to the main thread BEFORE driving
    # the stream with block_until_ready, so the execute + result fetch get
    # enqueued while the uploads are still in flight -- the server runs the
    # kernel the moment the last transfer lands, hiding both round trips
    # under the streaming window (~80 ms).
    devices = runner["devices"]
    ch = _queue.Queue()

    def _put_shard(c):
        sh = x_flat[c * N_PER_CORE:(c + 1) * N_PER_CORE]
        g = jax.device_put(sh.astype(ml_dtypes.bfloat16), devices[c])
        ch.put((c, g))
        g.block_until_ready()

    futs = [runner["pool"].submit(_put_shard, c) for c in range(N_CORES)]

    # overlap with the streams: codebook check/upload + on-device zeros
    codebook = np.ascontiguousarray(np.asarray(codebook, dtype=np.float32))
    cb = _get_cb_arrays(codebook, runner)
    zeros_g = runner["zeros_fn"]()  # async, on-device

    got = {}
    while len(got) < N_CORES:
        c, g = ch.get()
        got[c] = g
    x_g = jax.make_array_from_single_device_arrays(
        (B * S, D), runner["sharding"], [got[c] for c in range(N_CORES)])

    by_name = {"xn": x_g, "et": cb["et"], "ne2": cb["ne2"], "sel": cb["sel"],
               "ident": cb["ident"], "offs3": cb["offs3"]}
    args = [by_name[n] for n in runner["in_names"]] + [zeros_g]
    (out_g,) = runner["jitted"](*args)            # args may still be in flight
    out = np.asarray(out_g)                       # [8*128, 4*T_TILES] f32
    for f in futs:
        f.result()                                # surface upload errors

    per_core = out.reshape(N_CORES, 128, 4 * T_TILES)

    def blk(i):  # token i within core = t*128 + p
        return (per_core[:, :, i * T_TILES:(i + 1) * T_TILES]
                .transpose(0, 2, 1).reshape(-1))

    idx = blk(0).astype(np.int64)
    idx2 = blk(1).astype(np.int64)
    g12 = blk(2)
    g13 = blk(3)

    # bf16 upload perturbs scores by sigma ~ 0.058; re-check near-ties with
    # exact f32 math.  gap12 < theta: decide between the device's top-2 by
    # exact score (cheap).  gap13 also < theta (or degenerate tie): full
    # exact rescan of that token against all K codes.
    flag = g12 < GAP_THETA
    full = flag & ((g13 < GAP_THETA) | (g12 <= 0) | (idx2 >= K))
    cheap = np.nonzero(flag & ~full)[0]
    if cheap.size:
        i1 = idx[cheap]
        i2 = idx2[cheap]
        xs = x_flat[cheap]
        s1 = 2.0 * np.einsum("nd,nd->n", xs, codebook[i1]) - cb["e2"][i1]
        s2 = 2.0 * np.einsum("nd,nd->n", xs, codebook[i2]) - cb["e2"][i2]
        take2 = (s2 > s1) | ((s2 == s1) & (i2 < i1))
        idx[cheap] = np.where(take2, i2, i1)
    full_i = np.nonzero(full)[0]
    if full_i.size:
        sc = x_flat[full_i] @ cb["cbT2"]
        sc -= cb["e2"]
        idx[full_i] = sc.argmax(axis=1)

    # mode='clip' + preallocated out skips bounds checks and the fancy-index
    # temp allocation (13 ms vs 17.5 ms); idx is always in [0, K) here
    qout = np.empty((B * S, D), np.float32)
    np.take(codebook, idx, axis=0, mode="clip", out=qout)
    return qout.reshape(B, S, D).astype(x.dtype, copy=False)
